# revision 6
# baseline (speedup 1.0000x reference)
"""Trainium2 Bass kernel for the CombinedCriterionAE loss (retrieval_knn).

Math (see module docstring of the original nn.Module):
    loss = 0.45 * reg_loss + 0.45 * mean_i(1 - cos(pred_u[i], gt_u[argmin_j d2[i,j]]))
    d2[i,j] = |p_i|^2 + |g_j|^2 - 2 p_i.g_j   (argmin over j = 32768 gt points)

Distribution: pred rows (16384) sharded 8 ways (2048/core); gt replicated.

Per-core device algorithm:
  - scores s[i,j] = 2 p_i.g_j - |g_j|^2 (argmax of s == argmin of d2, per row)
    computed on the PE as a K=14 fp16 matmul using an exact hi/lo fp16 split
    of the fp32 operands (all partial products are exact in fp32, so the
    result carries fp32-level precision at 1 cycle/row instead of fp32's 4).
  - DVE Max per 1024-wide chunk -> per-chunk top value (the single full
    pass over all scores; this is the bottleneck engine).
  - Max/MaxIndex over the 32 chunk-maxes -> winning chunk per row.
  - indirect-DMA gather of the winning chunk's raw gt data (per-partition
    row gather), tiny fp32 recompute + MaxIndex -> index within chunk.
  - indirect-DMA gather of the (pre-normalized) gt normal, fused dot +
    accumulate -> per-partition running sum of cos values.
Host: shard/unshard, operand layout prep (fp16 splits, chunk table,
normal normalization), final scalar assembly (tiny reg_loss + mean).
"""

import sys

sys.path.insert(0, "/opt/trn_rl_repo")

import numpy as np

import concourse.bacc as bacc
import concourse.mybir as mybir
from concourse.bass import IndirectOffsetOnAxis
from concourse.bass_utils import run_bass_kernel_spmd
from concourse.tile import TileContext

BETA = 0.45
GAMMA = 0.45

N_PRED = 16384
N_GT = 32768
N_CORES = 8
NP_CORE = N_PRED // N_CORES      # 2048 pred rows per core
P = 128                          # partitions
N_TILES = NP_CORE // P           # 16 pred tiles per core
W = 1024                         # gt chunk width for the chunk-max pass
N_CHUNKS = N_GT // W             # 32
K = 14                           # matmul contraction: 4 per coord + 2 for g2

f32 = mybir.dt.float32
f16 = mybir.dt.float16
u32 = mybir.dt.uint32
i32 = mybir.dt.int32

_COMPILED = None  # (nc,) cache


def _build_bass():
    nc = bacc.Bacc(None, target_bir_lowering=False)

    predT_d = nc.dram_tensor("predt16", [K, NP_CORE], f16, kind="ExternalInput")
    gt16_d = nc.dram_tensor("gt16", [K, N_GT], f16, kind="ExternalInput")
    gtch_d = nc.dram_tensor("gtchunks", [N_CHUNKS, 4 * W], f32, kind="ExternalInput")
    gtn4_d = nc.dram_tensor("gtn4", [N_GT, 4], f32, kind="ExternalInput")
    ps_d = nc.dram_tensor("ps", [P, 4 * N_TILES], f32, kind="ExternalInput")
    pnu_d = nc.dram_tensor("pnu", [P, 4 * N_TILES], f32, kind="ExternalInput")
    acc_out = nc.dram_tensor("acc_out", [P], f32, kind="ExternalOutput")
    idx_out = nc.dram_tensor("idx_out", [NP_CORE], i32, kind="ExternalOutput")

    idx_out_t = idx_out[:].rearrange("(t p) -> t p", p=P)

    add = mybir.AluOpType.add
    sub = mybir.AluOpType.subtract
    mult = mybir.AluOpType.mult

    with TileContext(nc) as tc:
        with (
            tc.tile_pool(name="consts", bufs=1) as cpool,
            tc.tile_pool(name="psum", bufs=4, space="PSUM") as ppool,
            tc.tile_pool(name="cm", bufs=2) as cmpool,
            tc.tile_pool(name="gath", bufs=2) as gpool,
            tc.tile_pool(name="sc", bufs=2) as spool,
            tc.tile_pool(name="small", bufs=4) as mpool,
            tc.tile_pool(name="accp", bufs=2) as apool,
        ):
            gt16_s = cpool.tile([K, N_GT], f16, tag="gt16")
            predT_s = cpool.tile([K, NP_CORE], f16, tag="predt")
            ps_s = cpool.tile([P, 4 * N_TILES], f32, tag="ps")
            pnu_s = cpool.tile([P, 4 * N_TILES], f32, tag="pnu")
            nc.sync.dma_start(out=gt16_s[:], in_=gt16_d[:])
            nc.sync.dma_start(out=predT_s[:], in_=predT_d[:])
            nc.sync.dma_start(out=ps_s[:], in_=ps_d[:])
            nc.sync.dma_start(out=pnu_s[:], in_=pnu_d[:])

            acc_prev = None
            for t in range(N_TILES):
                lhsT = predT_s[:, t * P : (t + 1) * P]

                # --- chunk-max pass over all gt ---
                cm8 = cmpool.tile([P, 8 * N_CHUNKS], f32, tag="cm8")
                for c in range(N_CHUNKS):
                    pt = ppool.tile([P, W], f32, tag="score")
                    for h in range(W // 512):
                        nc.tensor.matmul(
                            out=pt[:, h * 512 : (h + 1) * 512],
                            lhsT=lhsT,
                            rhs=gt16_s[:, c * W + h * 512 : c * W + (h + 1) * 512],
                            start=True,
                            stop=True,
                        )
                    nc.vector.max(cm8[:, c * 8 : (c + 1) * 8], pt[:])

                # --- winning chunk per row ---
                cmv = cm8[:, 0 : 8 * N_CHUNKS : 8]  # [P, N_CHUNKS] strided view
                gm8 = mpool.tile([P, 8], f32, tag="gm8")
                ci8 = mpool.tile([P, 8], u32, tag="ci8")
                nc.vector.max(gm8[:], cmv)
                nc.vector.max_index(ci8[:], gm8[:], cmv)

                # --- gather winning chunk's raw gt data, one row per partition ---
                gath = gpool.tile([P, 4 * W], f32, tag="gath")
                nc.gpsimd.indirect_dma_start(
                    out=gath[:],
                    out_offset=None,
                    in_=gtch_d[:],
                    in_offset=IndirectOffsetOnAxis(ap=ci8[:, 0:1], axis=0),
                )

                # --- fp32 recompute of the winning chunk + index within chunk ---
                s = spool.tile([P, W], f32, tag="s")
                px2 = ps_s[:, 4 * t + 0 : 4 * t + 1]
                py2 = ps_s[:, 4 * t + 1 : 4 * t + 2]
                pz2 = ps_s[:, 4 * t + 2 : 4 * t + 3]
                nc.vector.scalar_tensor_tensor(
                    out=s[:], in0=gath[:, 0:W], scalar=px2,
                    in1=gath[:, 3 * W : 4 * W], op0=mult, op1=sub,
                )
                nc.vector.scalar_tensor_tensor(
                    out=s[:], in0=gath[:, W : 2 * W], scalar=py2,
                    in1=s[:], op0=mult, op1=add,
                )
                nc.vector.scalar_tensor_tensor(
                    out=s[:], in0=gath[:, 2 * W : 3 * W], scalar=pz2,
                    in1=s[:], op0=mult, op1=add,
                )
                w8 = mpool.tile([P, 8], f32, tag="w8")
                li8 = mpool.tile([P, 8], u32, tag="li8")
                nc.vector.max(w8[:], s[:])
                nc.vector.max_index(li8[:], w8[:], s[:])

                # --- global index = chunk*W + local (exact via f32) ---
                cif = mpool.tile([P, 1], f32, tag="cif")
                lif = mpool.tile([P, 1], f32, tag="lif")
                gif = mpool.tile([P, 1], f32, tag="gif")
                gidx = mpool.tile([P, 1], u32, tag="gidx")
                gidx_i = mpool.tile([P, 1], i32, tag="gidxi")
                nc.vector.tensor_copy(cif[:], ci8[:, 0:1])
                nc.vector.tensor_copy(lif[:], li8[:, 0:1])
                nc.vector.scalar_tensor_tensor(
                    out=gif[:], in0=cif[:], scalar=float(W), in1=lif[:],
                    op0=mult, op1=add,
                )
                nc.vector.tensor_copy(gidx[:], gif[:])
                nc.vector.tensor_copy(gidx_i[:], gif[:])
                nc.sync.dma_start(out=idx_out_t[t], in_=gidx_i[:, 0])

                # --- gather pre-normalized gt normal; fused dot + accumulate ---
                gn = mpool.tile([P, 4], f32, tag="gn")
                nc.gpsimd.indirect_dma_start(
                    out=gn[:],
                    out_offset=None,
                    in_=gtn4_d[:],
                    in_offset=IndirectOffsetOnAxis(ap=gidx[:, 0:1], axis=0),
                )
                prod = mpool.tile([P, 3], f32, tag="prod")
                dot = mpool.tile([P, 1], f32, tag="dot")
                acc = apool.tile([P, 1], f32, tag="acc")
                nc.vector.tensor_tensor(
                    out=prod[:], in0=pnu_s[:, 4 * t : 4 * t + 3],
                    in1=gn[:, 0:3], op=mult,
                )
                nc.vector.reduce_sum(dot[:], prod[:], axis=mybir.AxisListType.X)
                if acc_prev is None:
                    nc.vector.tensor_copy(acc[:], dot[:])
                else:
                    nc.vector.tensor_tensor(
                        out=acc[:], in0=acc_prev[:], in1=dot[:], op=add,
                    )
                acc_prev = acc

            nc.sync.dma_start(out=acc_out[:], in_=acc_prev[:, 0])

    nc.finalize()
    return nc


def _split16(x32: np.ndarray):
    """Exact fp16 hi/lo split: x32 ~= hi + lo with error ~2^-22 relative."""
    hi = x32.astype(np.float16)
    lo = (x32 - hi.astype(np.float32)).astype(np.float16)
    return hi, lo


def _prep_inputs(pred_feat: np.ndarray, gt_data: np.ndarray):
    """Host-side layout marshalling (O(N+L) work only)."""
    Pp = pred_feat[:, :3].astype(np.float32)
    PN = pred_feat[:, 3:].astype(np.float32)
    G = gt_data[:, :3].astype(np.float32)
    GN = gt_data[:, 3:].astype(np.float32)

    P2 = (2.0 * Pp).astype(np.float32)
    ph, pl = _split16(P2)
    gh, gl = _split16(G)
    g2 = (G[:, 0] * G[:, 0] + G[:, 1] * G[:, 1] + G[:, 2] * G[:, 2]).astype(np.float32)
    g2h, g2l = _split16(g2)

    # K=14 matmul operands (exact products: hi*hi, hi*lo, lo*hi, lo*lo per coord)
    predT = np.empty((K, N_PRED), np.float16)
    rhs = np.empty((K, N_GT), np.float16)
    r = 0
    for c in range(3):
        for a, b in ((ph, gh), (ph, gl), (pl, gh), (pl, gl)):
            predT[r] = a[:, c]
            rhs[r] = b[:, c]
            r += 1
    predT[12] = np.float16(-1.0)
    rhs[12] = g2h
    predT[13] = np.float16(-1.0)
    rhs[13] = g2l

    # raw f32 chunk table for the winning-chunk recompute gather
    gtch = np.empty((N_CHUNKS, 4 * W), np.float32)
    for c in range(N_CHUNKS):
        sl = slice(c * W, (c + 1) * W)
        gtch[c, 0:W] = G[sl, 0]
        gtch[c, W : 2 * W] = G[sl, 1]
        gtch[c, 2 * W : 3 * W] = G[sl, 2]
        gtch[c, 3 * W : 4 * W] = g2[sl]

    def _l2n(x):
        n = np.linalg.norm(x, axis=-1, keepdims=True)
        return x / np.maximum(n, 1e-12)

    gtn4 = np.zeros((N_GT, 4), np.float32)
    gtn4[:, 0:3] = _l2n(GN)
    pu = _l2n(PN).astype(np.float32)

    in_maps = []
    for core in range(N_CORES):
        rows = slice(core * NP_CORE, (core + 1) * NP_CORE)
        # per-partition scalar layout: [p, 4*t + k], global row = core*2048 + t*128 + p
        ps = np.zeros((P, 4 * N_TILES), np.float32)
        pnu = np.zeros((P, 4 * N_TILES), np.float32)
        p2c = P2[rows].reshape(N_TILES, P, 3)
        puc = pu[rows].reshape(N_TILES, P, 3)
        for t in range(N_TILES):
            ps[:, 4 * t : 4 * t + 3] = p2c[t]
            pnu[:, 4 * t : 4 * t + 3] = puc[t]
        in_maps.append(
            {
                "predt16": np.ascontiguousarray(predT[:, rows]),
                "gt16": rhs,
                "gtchunks": gtch,
                "gtn4": gtn4,
                "ps": ps,
                "pnu": pnu,
            }
        )
    return in_maps


def _get_nc():
    global _COMPILED
    if _COMPILED is None:
        _COMPILED = _build_bass()
    return _COMPILED


def _run_maps(in_maps, trace=False, **trace_kwargs):
    return run_bass_kernel_spmd(
        _get_nc(), in_maps, list(range(N_CORES)), trace=trace, **trace_kwargs
    )


def _run(pred_feat, gt_data, trace=False, **trace_kwargs):
    return _run_maps(_prep_inputs(pred_feat, gt_data), trace=trace, **trace_kwargs)


def kernel(pred_feat, gt_data, R, t, s) -> np.ndarray:
    pred_feat = np.asarray(pred_feat, np.float32)
    gt_data = np.asarray(gt_data, np.float32)
    R = np.asarray(R, np.float32)
    t = np.asarray(t, np.float32)
    s = np.asarray(s, np.float32)

    res = _run(pred_feat, gt_data)

    cos_sum = np.float64(0.0)
    for core in range(N_CORES):
        cos_sum += np.float32(res.results[core]["acc_out"].sum())
    norm_loss = np.float32(1.0 - np.float32(cos_sum) / np.float32(N_PRED))

    reg_loss = (
        np.linalg.norm(R - np.eye(3, dtype=np.float32))
        + np.linalg.norm(t)
        + (s[0] - np.float32(1.0)) ** 2
    )
    return np.asarray(
        np.float32(BETA) * np.float32(reg_loss) + np.float32(GAMMA) * norm_loss,
        dtype=np.float32,
    )


# revision 9
# speedup vs baseline: 430.4244x; 430.4244x over previous
"""Trainium2 Bass kernel for the CombinedCriterionAE loss (retrieval_knn).

Math (see module docstring of the original nn.Module):
    loss = 0.45 * reg_loss + 0.45 * mean_i(1 - cos(pred_u[i], gt_u[argmin_j d2[i,j]]))
    d2[i,j] = |p_i|^2 + |g_j|^2 - 2 p_i.g_j   (argmin over j = 32768 gt points)

Distribution: pred rows (16384) sharded 8 ways (2048/core); gt replicated.

Per-core device algorithm:
  - scores s[i,j] = 2 p_i.g_j - |g_j|^2 (argmax of s == argmin of d2, per row)
    computed on the PE as a K=14 fp16 matmul using an exact hi/lo fp16 split
    of the fp32 operands (all partial products are exact in fp32, so the
    result carries fp32-level precision at 1 cycle/row instead of fp32's 4).
  - DVE Max per 1024-wide chunk directly on PSUM -> per-chunk top value
    (the single full pass over all scores; DVE is the bottleneck engine).
  - Max/MaxIndex over the 32 chunk-maxes -> winning chunk per row.
  - indirect-DMA gather of the winning chunk's raw gt data (per-partition
    row gather), tiny fp32 recompute + MaxIndex -> index within chunk.
  - indirect-DMA gather of the (pre-normalized) gt normal, dot + accumulate
    -> per-partition running sum of cos values.
Host: shard/unshard, operand layout prep (fp16 splits, chunk table,
normal normalization), final scalar assembly (tiny reg_loss + mean).
"""

import sys

sys.path.insert(0, "/opt/trn_rl_repo")

import numpy as np

import concourse.bacc as bacc
import concourse.mybir as mybir
from concourse.bass import IndirectOffsetOnAxis
from concourse.bass_utils import run_bass_kernel_spmd
from concourse.tile import TileContext

BETA = 0.45
GAMMA = 0.45

N_PRED = 16384
N_GT = 32768
N_CORES = 8
NP_CORE = N_PRED // N_CORES      # 2048 pred rows per core
P = 128                          # partitions
N_TILES = NP_CORE // P           # 16 pred tiles per core
W = 1024                         # gt chunk width for the chunk-max pass
N_CHUNKS = N_GT // W             # 32
K = 14                           # matmul contraction: 4 per coord + 2 for g2

f32 = mybir.dt.float32
f16 = mybir.dt.float16
u32 = mybir.dt.uint32
i32 = mybir.dt.int32

_COMPILED = {}  # repeat -> nc


def _build_bass(repeat=1):
    nc = bacc.Bacc(None, target_bir_lowering=False)

    predT_d = nc.dram_tensor("predt16", [K, NP_CORE], f16, kind="ExternalInput")
    gt16_d = nc.dram_tensor("gt16", [K, N_GT], f16, kind="ExternalInput")
    gtch_d = nc.dram_tensor("gtchunks", [N_CHUNKS, 4 * W], f32, kind="ExternalInput")
    gtn4_d = nc.dram_tensor("gtn4", [N_GT, 4], f32, kind="ExternalInput")
    ps_d = nc.dram_tensor("ps", [P, 4 * N_TILES], f32, kind="ExternalInput")
    pnu_d = nc.dram_tensor("pnu", [P, 4 * N_TILES], f32, kind="ExternalInput")
    acc_out = nc.dram_tensor("acc_out", [P], f32, kind="ExternalOutput")
    idx_out = nc.dram_tensor("idx_out", [NP_CORE], i32, kind="ExternalOutput")

    idx_out_t = idx_out[:].rearrange("(t p) -> t p", p=P)

    add = mybir.AluOpType.add
    sub = mybir.AluOpType.subtract
    mult = mybir.AluOpType.mult

    with TileContext(nc) as tc:
        with (
            tc.tile_pool(name="consts", bufs=1) as cpool,
            tc.tile_pool(name="psum", bufs=4, space="PSUM") as ppool,
            tc.tile_pool(name="cm", bufs=2) as cmpool,
            tc.tile_pool(name="gath", bufs=2) as gpool,
            tc.tile_pool(name="sc", bufs=2) as spool,
            tc.tile_pool(name="small", bufs=4) as mpool,
            tc.tile_pool(name="accp", bufs=2) as apool,
        ):
            gt16_s = cpool.tile([K, N_GT], f16, tag="gt16")
            predT_s = cpool.tile([K, NP_CORE], f16, tag="predt")
            ps_s = cpool.tile([P, 4 * N_TILES], f32, tag="ps")
            pnu_s = cpool.tile([P, 4 * N_TILES], f32, tag="pnu")
            nc.sync.dma_start(out=gt16_s[:], in_=gt16_d[:])
            nc.sync.dma_start(out=predT_s[:], in_=predT_d[:])
            nc.sync.dma_start(out=ps_s[:], in_=ps_d[:])
            nc.sync.dma_start(out=pnu_s[:], in_=pnu_d[:])

            def body():
                acc_prev = None
                for t in range(N_TILES):
                    lhsT = predT_s[:, t * P : (t + 1) * P]

                    # --- chunk-max pass over all gt ---
                    cm8 = cmpool.tile([P, 8 * N_CHUNKS], f32, tag="cm8")
                    for c in range(N_CHUNKS):
                        pt = ppool.tile([P, W], f32, tag="score")
                        for h in range(W // 512):
                            nc.tensor.matmul(
                                out=pt[:, h * 512 : (h + 1) * 512],
                                lhsT=lhsT,
                                rhs=gt16_s[
                                    :, c * W + h * 512 : c * W + (h + 1) * 512
                                ],
                                start=True,
                                stop=True,
                            )
                        nc.vector.max(cm8[:, c * 8 : (c + 1) * 8], pt[:])

                    # --- winning chunk per row ---
                    cmv = cm8[:, 0 : 8 * N_CHUNKS : 8]  # [P, N_CHUNKS] stride-8
                    gm8 = mpool.tile([P, 8], f32, tag="gm8")
                    ci8 = mpool.tile([P, 8], u32, tag="ci8")
                    nc.vector.max(gm8[:], cmv)
                    nc.vector.max_index(ci8[:], gm8[:], cmv)

                    # --- gather winning chunk's raw gt data (row per partition) ---
                    gath = gpool.tile([P, 4 * W], f32, tag="gath")
                    nc.gpsimd.indirect_dma_start(
                        out=gath[:],
                        out_offset=None,
                        in_=gtch_d[:],
                        in_offset=IndirectOffsetOnAxis(ap=ci8[:, 0:1], axis=0),
                    )

                    # --- fp32 recompute of winning chunk + index within chunk ---
                    s = spool.tile([P, W], f32, tag="s")
                    px2 = ps_s[:, 4 * t + 0 : 4 * t + 1]
                    py2 = ps_s[:, 4 * t + 1 : 4 * t + 2]
                    pz2 = ps_s[:, 4 * t + 2 : 4 * t + 3]
                    nc.vector.scalar_tensor_tensor(
                        out=s[:], in0=gath[:, 0:W], scalar=px2,
                        in1=gath[:, 3 * W : 4 * W], op0=mult, op1=sub,
                    )
                    nc.vector.scalar_tensor_tensor(
                        out=s[:], in0=gath[:, W : 2 * W], scalar=py2,
                        in1=s[:], op0=mult, op1=add,
                    )
                    nc.vector.scalar_tensor_tensor(
                        out=s[:], in0=gath[:, 2 * W : 3 * W], scalar=pz2,
                        in1=s[:], op0=mult, op1=add,
                    )
                    w8 = mpool.tile([P, 8], f32, tag="w8")
                    li8 = mpool.tile([P, 8], u32, tag="li8")
                    nc.vector.max(w8[:], s[:])
                    nc.vector.max_index(li8[:], w8[:], s[:])

                    # --- global index = chunk*W + local (exact via f32) ---
                    cif = mpool.tile([P, 1], f32, tag="cif")
                    lif = mpool.tile([P, 1], f32, tag="lif")
                    gif = mpool.tile([P, 1], f32, tag="gif")
                    gidx = mpool.tile([P, 1], u32, tag="gidx")
                    gidx_i = mpool.tile([P, 1], i32, tag="gidxi")
                    nc.vector.tensor_copy(cif[:], ci8[:, 0:1])
                    nc.vector.tensor_copy(lif[:], li8[:, 0:1])
                    nc.vector.scalar_tensor_tensor(
                        out=gif[:], in0=cif[:], scalar=float(W), in1=lif[:],
                        op0=mult, op1=add,
                    )
                    nc.vector.tensor_copy(gidx[:], gif[:])
                    nc.vector.tensor_copy(gidx_i[:], gif[:])
                    nc.sync.dma_start(out=idx_out_t[t], in_=gidx_i[:, 0])

                    # --- gather pre-normalized gt normal; dot + accumulate ---
                    gn = mpool.tile([P, 4], f32, tag="gn")
                    nc.gpsimd.indirect_dma_start(
                        out=gn[:],
                        out_offset=None,
                        in_=gtn4_d[:],
                        in_offset=IndirectOffsetOnAxis(ap=gidx[:, 0:1], axis=0),
                    )
                    prod = mpool.tile([P, 3], f32, tag="prod")
                    dot = mpool.tile([P, 1], f32, tag="dot")
                    acc = apool.tile([P, 1], f32, tag="acc")
                    nc.vector.tensor_tensor(
                        out=prod[:], in0=pnu_s[:, 4 * t : 4 * t + 3],
                        in1=gn[:, 0:3], op=mult,
                    )
                    nc.vector.reduce_sum(dot[:], prod[:], axis=mybir.AxisListType.X)
                    if acc_prev is None:
                        nc.vector.tensor_copy(acc[:], dot[:])
                    else:
                        nc.vector.tensor_tensor(
                            out=acc[:], in0=acc_prev[:], in1=dot[:], op=add,
                        )
                    acc_prev = acc

                nc.sync.dma_start(out=acc_out[:], in_=acc_prev[:, 0])

            if repeat > 1:
                with tc.For_i(0, repeat, 1):
                    body()
            else:
                body()

    nc.finalize()
    return nc


def _split16(x32: np.ndarray):
    """Exact fp16 hi/lo split: x32 ~= hi + lo with error ~2^-22 relative."""
    hi = x32.astype(np.float16)
    lo = (x32 - hi.astype(np.float32)).astype(np.float16)
    return hi, lo


def _prep_inputs(pred_feat: np.ndarray, gt_data: np.ndarray):
    """Host-side layout marshalling (O(N+L) work only)."""
    Pp = pred_feat[:, :3].astype(np.float32)
    PN = pred_feat[:, 3:].astype(np.float32)
    G = gt_data[:, :3].astype(np.float32)
    GN = gt_data[:, 3:].astype(np.float32)

    P2 = (2.0 * Pp).astype(np.float32)
    ph, pl = _split16(P2)
    gh, gl = _split16(G)
    g2 = (G[:, 0] * G[:, 0] + G[:, 1] * G[:, 1] + G[:, 2] * G[:, 2]).astype(np.float32)
    g2h, g2l = _split16(g2)

    # K=14 matmul operands (exact products: hi*hi, hi*lo, lo*hi, lo*lo per coord)
    predT = np.empty((K, N_PRED), np.float16)
    rhs = np.empty((K, N_GT), np.float16)
    r = 0
    for c in range(3):
        for a, b in ((ph, gh), (ph, gl), (pl, gh), (pl, gl)):
            predT[r] = a[:, c]
            rhs[r] = b[:, c]
            r += 1
    predT[12] = np.float16(-1.0)
    rhs[12] = g2h
    predT[13] = np.float16(-1.0)
    rhs[13] = g2l

    # raw f32 chunk table for the winning-chunk recompute gather
    gtch = np.empty((N_CHUNKS, 4 * W), np.float32)
    for c in range(N_CHUNKS):
        sl = slice(c * W, (c + 1) * W)
        gtch[c, 0:W] = G[sl, 0]
        gtch[c, W : 2 * W] = G[sl, 1]
        gtch[c, 2 * W : 3 * W] = G[sl, 2]
        gtch[c, 3 * W : 4 * W] = g2[sl]

    def _l2n(x):
        n = np.linalg.norm(x, axis=-1, keepdims=True)
        return x / np.maximum(n, 1e-12)

    gtn4 = np.zeros((N_GT, 4), np.float32)
    gtn4[:, 0:3] = _l2n(GN)
    pu = _l2n(PN).astype(np.float32)

    in_maps = []
    for core in range(N_CORES):
        rows = slice(core * NP_CORE, (core + 1) * NP_CORE)
        # per-partition scalar layout: [p, 4*t + k], global row = core*2048 + t*128 + p
        ps = np.zeros((P, 4 * N_TILES), np.float32)
        pnu = np.zeros((P, 4 * N_TILES), np.float32)
        p2c = P2[rows].reshape(N_TILES, P, 3)
        puc = pu[rows].reshape(N_TILES, P, 3)
        for t in range(N_TILES):
            ps[:, 4 * t : 4 * t + 3] = p2c[t]
            pnu[:, 4 * t : 4 * t + 3] = puc[t]
        in_maps.append(
            {
                "predt16": np.ascontiguousarray(predT[:, rows]),
                "gt16": rhs,
                "gtchunks": gtch,
                "gtn4": gtn4,
                "ps": ps,
                "pnu": pnu,
            }
        )
    return in_maps


def _get_nc(repeat=1):
    if repeat not in _COMPILED:
        _COMPILED[repeat] = _build_bass(repeat)
    return _COMPILED[repeat]


def _run_maps(in_maps, repeat=1, trace=False, **trace_kwargs):
    return run_bass_kernel_spmd(
        _get_nc(repeat), in_maps, list(range(N_CORES)), trace=trace, **trace_kwargs
    )


def _run(pred_feat, gt_data, trace=False, **trace_kwargs):
    return _run_maps(_prep_inputs(pred_feat, gt_data), trace=trace, **trace_kwargs)


def kernel(pred_feat, gt_data, R, t, s) -> np.ndarray:
    pred_feat = np.asarray(pred_feat, np.float32)
    gt_data = np.asarray(gt_data, np.float32)
    R = np.asarray(R, np.float32)
    t = np.asarray(t, np.float32)
    s = np.asarray(s, np.float32)

    res = _run(pred_feat, gt_data)

    cos_sum = np.float64(0.0)
    for core in range(N_CORES):
        cos_sum += np.float32(res.results[core]["acc_out"].sum())
    norm_loss = np.float32(1.0 - np.float32(cos_sum) / np.float32(N_PRED))

    reg_loss = (
        np.linalg.norm(R - np.eye(3, dtype=np.float32))
        + np.linalg.norm(t)
        + (s[0] - np.float32(1.0)) ** 2
    )
    return np.asarray(
        np.float32(BETA) * np.float32(reg_loss) + np.float32(GAMMA) * norm_loss,
        dtype=np.float32,
    )


# revision 11
# speedup vs baseline: 585.4085x; 1.3601x over previous
"""Trainium2 Bass kernel for the CombinedCriterionAE loss (retrieval_knn).

Math (see module docstring of the original nn.Module):
    loss = 0.45 * reg_loss + 0.45 * mean_i(1 - cos(pred_u[i], gt_u[argmin_j d2[i,j]]))
    d2[i,j] = |p_i|^2 + |g_j|^2 - 2 p_i.g_j   (argmin over j = 32768 gt points)

Distribution: pred rows (16384) sharded 8 ways (2048/core); gt replicated.

Per-core device algorithm:
  - scores s[i,j] = 2 p_i.g_j - |g_j|^2 (argmax of s == argmin of d2, per row)
    computed on the PE as a K=14 fp16 matmul using an exact hi/lo fp16 split
    of the fp32 operands (all partial products are exact in fp32, so the
    result carries fp32-level precision at 1 cycle/row instead of fp32's 4).
  - DVE Max per 1024-wide chunk directly on PSUM -> per-chunk top value
    (the single full pass over all scores; DVE is the bottleneck engine).
  - Max/MaxIndex over the 32 chunk-maxes -> winning chunk per row.
  - indirect-DMA gather of the winning chunk's raw gt data (per-partition
    row gather), tiny fp32 recompute + MaxIndex -> index within chunk.
  - indirect-DMA gather of the (pre-normalized) gt normal, dot + accumulate
    -> per-partition running sum of cos values.
Host: shard/unshard, operand layout prep (fp16 splits, chunk table,
normal normalization), final scalar assembly (tiny reg_loss + mean).
"""

import sys

sys.path.insert(0, "/opt/trn_rl_repo")

import numpy as np

import concourse.bacc as bacc
import concourse.mybir as mybir
from concourse.bass import IndirectOffsetOnAxis
from concourse.bass_utils import run_bass_kernel_spmd
from concourse.tile import TileContext

BETA = 0.45
GAMMA = 0.45

N_PRED = 16384
N_GT = 32768
N_CORES = 8
NP_CORE = N_PRED // N_CORES      # 2048 pred rows per core
P = 128                          # partitions
N_TILES = NP_CORE // P           # 16 pred tiles per core
W = 1024                         # gt chunk width for the chunk-max pass
N_CHUNKS = N_GT // W             # 32
K = 14                           # matmul contraction: 4 per coord + 2 for g2

f32 = mybir.dt.float32
f16 = mybir.dt.float16
u32 = mybir.dt.uint32
i32 = mybir.dt.int32

_COMPILED = {}  # repeat -> nc


def _build_bass(repeat=1):
    nc = bacc.Bacc(None, target_bir_lowering=False)

    predT_d = nc.dram_tensor("predt16", [K, NP_CORE], f16, kind="ExternalInput")
    gt16_d = nc.dram_tensor("gt16", [K, N_GT], f16, kind="ExternalInput")
    gtch_d = nc.dram_tensor("gtchunks", [N_CHUNKS, 4 * W], f32, kind="ExternalInput")
    gtn4_d = nc.dram_tensor("gtn4", [N_GT, 4], f32, kind="ExternalInput")
    ps_d = nc.dram_tensor("ps", [P, 4 * N_TILES], f32, kind="ExternalInput")
    pnu_d = nc.dram_tensor("pnu", [P, 4 * N_TILES], f32, kind="ExternalInput")
    acc_out = nc.dram_tensor("acc_out", [P], f32, kind="ExternalOutput")
    idx_out = nc.dram_tensor("idx_out", [NP_CORE], i32, kind="ExternalOutput")

    idx_out_t = idx_out[:].rearrange("(t p) -> t p", p=P)

    add = mybir.AluOpType.add
    sub = mybir.AluOpType.subtract
    mult = mybir.AluOpType.mult

    with TileContext(nc) as tc:
        with (
            tc.tile_pool(name="consts", bufs=1) as cpool,
            tc.tile_pool(name="psum", bufs=4, space="PSUM") as ppool,
            tc.tile_pool(name="cm", bufs=2) as cmpool,
            tc.tile_pool(name="gath", bufs=3) as gpool,
            tc.tile_pool(name="sc", bufs=2) as spool,
            tc.tile_pool(name="small", bufs=4) as mpool,
            tc.tile_pool(name="accp", bufs=2) as apool,
        ):
            gt16_s = cpool.tile([K, N_GT], f16, tag="gt16")
            predT_s = cpool.tile([K, NP_CORE], f16, tag="predt")
            ps_s = cpool.tile([P, 4 * N_TILES], f32, tag="ps")
            pnu_s = cpool.tile([P, 4 * N_TILES], f32, tag="pnu")
            nc.sync.dma_start(out=gt16_s[:], in_=gt16_d[:])
            nc.sync.dma_start(out=predT_s[:], in_=predT_d[:])
            nc.sync.dma_start(out=ps_s[:], in_=ps_d[:])
            nc.sync.dma_start(out=pnu_s[:], in_=pnu_d[:])

            def emit_chunkmax(t, st):
                """Bulk pass: matmul scores into PSUM, per-chunk Max, pick
                winning chunk, launch the chunk-data gather. Fills `st`."""
                lhsT = predT_s[:, t * P : (t + 1) * P]
                cm8 = cmpool.tile([P, 8 * N_CHUNKS], f32, tag="cm8")
                for c in range(N_CHUNKS):
                    pt = ppool.tile([P, W], f32, tag="score")
                    for h in range(W // 512):
                        nc.tensor.matmul(
                            out=pt[:, h * 512 : (h + 1) * 512],
                            lhsT=lhsT,
                            rhs=gt16_s[:, c * W + h * 512 : c * W + (h + 1) * 512],
                            start=True,
                            stop=True,
                        )
                    nc.vector.max(cm8[:, c * 8 : (c + 1) * 8], pt[:])

                cmv = cm8[:, 0 : 8 * N_CHUNKS : 8]  # [P, N_CHUNKS] stride-8
                gm8 = mpool.tile([P, 8], f32, tag="gm8")
                ci8 = mpool.tile([P, 8], u32, tag="ci8")
                nc.vector.max(gm8[:], cmv)
                nc.vector.max_index(ci8[:], gm8[:], cmv)

                gath = gpool.tile([P, 4 * W], f32, tag="gath")
                nc.gpsimd.indirect_dma_start(
                    out=gath[:],
                    out_offset=None,
                    in_=gtch_d[:],
                    in_offset=IndirectOffsetOnAxis(ap=ci8[:, 0:1], axis=0),
                )
                st["ci8"] = ci8
                st["gath"] = gath

            def emit_stage3a(t, st):
                """Recompute winning chunk in fp32, find index within chunk,
                form the global index, launch the normal gather."""
                gath = st["gath"]
                s = spool.tile([P, W], f32, tag="s")
                px2 = ps_s[:, 4 * t + 0 : 4 * t + 1]
                py2 = ps_s[:, 4 * t + 1 : 4 * t + 2]
                pz2 = ps_s[:, 4 * t + 2 : 4 * t + 3]
                nc.vector.scalar_tensor_tensor(
                    out=s[:], in0=gath[:, 0:W], scalar=px2,
                    in1=gath[:, 3 * W : 4 * W], op0=mult, op1=sub,
                )
                nc.vector.scalar_tensor_tensor(
                    out=s[:], in0=gath[:, W : 2 * W], scalar=py2,
                    in1=s[:], op0=mult, op1=add,
                )
                nc.vector.scalar_tensor_tensor(
                    out=s[:], in0=gath[:, 2 * W : 3 * W], scalar=pz2,
                    in1=s[:], op0=mult, op1=add,
                )
                w8 = mpool.tile([P, 8], f32, tag="w8")
                li8 = mpool.tile([P, 8], u32, tag="li8")
                nc.vector.max(w8[:], s[:])
                nc.vector.max_index(li8[:], w8[:], s[:])

                cif = mpool.tile([P, 1], f32, tag="cif")
                lif = mpool.tile([P, 1], f32, tag="lif")
                gif = mpool.tile([P, 1], f32, tag="gif")
                gidx = mpool.tile([P, 1], u32, tag="gidx")
                gidx_i = mpool.tile([P, 1], i32, tag="gidxi")
                nc.vector.tensor_copy(cif[:], st["ci8"][:, 0:1])
                nc.vector.tensor_copy(lif[:], li8[:, 0:1])
                nc.vector.scalar_tensor_tensor(
                    out=gif[:], in0=cif[:], scalar=float(W), in1=lif[:],
                    op0=mult, op1=add,
                )
                nc.vector.tensor_copy(gidx[:], gif[:])
                nc.vector.tensor_copy(gidx_i[:], gif[:])
                nc.sync.dma_start(out=idx_out_t[t], in_=gidx_i[:, 0])

                gn = mpool.tile([P, 4], f32, tag="gn")
                nc.gpsimd.indirect_dma_start(
                    out=gn[:],
                    out_offset=None,
                    in_=gtn4_d[:],
                    in_offset=IndirectOffsetOnAxis(ap=gidx[:, 0:1], axis=0),
                )
                st["gn"] = gn

            def emit_stage3b(t, st, acc_prev):
                """Dot with pre-normalized pred normal; accumulate cos sum."""
                prod = mpool.tile([P, 3], f32, tag="prod")
                dot = mpool.tile([P, 1], f32, tag="dot")
                acc = apool.tile([P, 1], f32, tag="acc")
                nc.vector.tensor_tensor(
                    out=prod[:], in0=pnu_s[:, 4 * t : 4 * t + 3],
                    in1=st["gn"][:, 0:3], op=mult,
                )
                nc.vector.reduce_sum(dot[:], prod[:], axis=mybir.AxisListType.X)
                if acc_prev is None:
                    nc.vector.tensor_copy(acc[:], dot[:])
                else:
                    nc.vector.tensor_tensor(
                        out=acc[:], in0=acc_prev[:], in1=dot[:], op=add,
                    )
                return acc

            def body():
                # Software pipeline with a 1/2-tile skew: while the DVE chews
                # tile t's 32 chunk-Maxes, tile t-1's chunk gather and tile
                # t-2's normal gather complete in the background, so the DVE's
                # in-order stream never stalls on DMA latency.
                acc_prev = None
                sts = [dict() for _ in range(N_TILES)]
                for t in range(N_TILES + 2):
                    if t < N_TILES:
                        emit_chunkmax(t, sts[t])
                    if 1 <= t < N_TILES + 1:
                        emit_stage3a(t - 1, sts[t - 1])
                    if t >= 2:
                        acc_prev = emit_stage3b(t - 2, sts[t - 2], acc_prev)

                nc.sync.dma_start(out=acc_out[:], in_=acc_prev[:, 0])

            if repeat > 1:
                with tc.For_i(0, repeat, 1):
                    body()
            else:
                body()

    nc.finalize()
    return nc


def _split16(x32: np.ndarray):
    """Exact fp16 hi/lo split: x32 ~= hi + lo with error ~2^-22 relative."""
    hi = x32.astype(np.float16)
    lo = (x32 - hi.astype(np.float32)).astype(np.float16)
    return hi, lo


def _prep_inputs(pred_feat: np.ndarray, gt_data: np.ndarray):
    """Host-side layout marshalling (O(N+L) work only)."""
    Pp = pred_feat[:, :3].astype(np.float32)
    PN = pred_feat[:, 3:].astype(np.float32)
    G = gt_data[:, :3].astype(np.float32)
    GN = gt_data[:, 3:].astype(np.float32)

    P2 = (2.0 * Pp).astype(np.float32)
    ph, pl = _split16(P2)
    gh, gl = _split16(G)
    g2 = (G[:, 0] * G[:, 0] + G[:, 1] * G[:, 1] + G[:, 2] * G[:, 2]).astype(np.float32)
    g2h, g2l = _split16(g2)

    # K=14 matmul operands (exact products: hi*hi, hi*lo, lo*hi, lo*lo per coord)
    predT = np.empty((K, N_PRED), np.float16)
    rhs = np.empty((K, N_GT), np.float16)
    r = 0
    for c in range(3):
        for a, b in ((ph, gh), (ph, gl), (pl, gh), (pl, gl)):
            predT[r] = a[:, c]
            rhs[r] = b[:, c]
            r += 1
    predT[12] = np.float16(-1.0)
    rhs[12] = g2h
    predT[13] = np.float16(-1.0)
    rhs[13] = g2l

    # raw f32 chunk table for the winning-chunk recompute gather
    gtch = np.empty((N_CHUNKS, 4 * W), np.float32)
    for c in range(N_CHUNKS):
        sl = slice(c * W, (c + 1) * W)
        gtch[c, 0:W] = G[sl, 0]
        gtch[c, W : 2 * W] = G[sl, 1]
        gtch[c, 2 * W : 3 * W] = G[sl, 2]
        gtch[c, 3 * W : 4 * W] = g2[sl]

    def _l2n(x):
        n = np.linalg.norm(x, axis=-1, keepdims=True)
        return x / np.maximum(n, 1e-12)

    gtn4 = np.zeros((N_GT, 4), np.float32)
    gtn4[:, 0:3] = _l2n(GN)
    pu = _l2n(PN).astype(np.float32)

    in_maps = []
    for core in range(N_CORES):
        rows = slice(core * NP_CORE, (core + 1) * NP_CORE)
        # per-partition scalar layout: [p, 4*t + k], global row = core*2048 + t*128 + p
        ps = np.zeros((P, 4 * N_TILES), np.float32)
        pnu = np.zeros((P, 4 * N_TILES), np.float32)
        p2c = P2[rows].reshape(N_TILES, P, 3)
        puc = pu[rows].reshape(N_TILES, P, 3)
        for t in range(N_TILES):
            ps[:, 4 * t : 4 * t + 3] = p2c[t]
            pnu[:, 4 * t : 4 * t + 3] = puc[t]
        in_maps.append(
            {
                "predt16": np.ascontiguousarray(predT[:, rows]),
                "gt16": rhs,
                "gtchunks": gtch,
                "gtn4": gtn4,
                "ps": ps,
                "pnu": pnu,
            }
        )
    return in_maps


def _get_nc(repeat=1):
    if repeat not in _COMPILED:
        _COMPILED[repeat] = _build_bass(repeat)
    return _COMPILED[repeat]


def _run_maps(in_maps, repeat=1, trace=False, **trace_kwargs):
    return run_bass_kernel_spmd(
        _get_nc(repeat), in_maps, list(range(N_CORES)), trace=trace, **trace_kwargs
    )


def _run(pred_feat, gt_data, trace=False, **trace_kwargs):
    return _run_maps(_prep_inputs(pred_feat, gt_data), trace=trace, **trace_kwargs)


def kernel(pred_feat, gt_data, R, t, s) -> np.ndarray:
    pred_feat = np.asarray(pred_feat, np.float32)
    gt_data = np.asarray(gt_data, np.float32)
    R = np.asarray(R, np.float32)
    t = np.asarray(t, np.float32)
    s = np.asarray(s, np.float32)

    res = _run(pred_feat, gt_data)

    cos_sum = np.float64(0.0)
    for core in range(N_CORES):
        cos_sum += np.float32(res.results[core]["acc_out"].sum())
    norm_loss = np.float32(1.0 - np.float32(cos_sum) / np.float32(N_PRED))

    reg_loss = (
        np.linalg.norm(R - np.eye(3, dtype=np.float32))
        + np.linalg.norm(t)
        + (s[0] - np.float32(1.0)) ** 2
    )
    return np.asarray(
        np.float32(BETA) * np.float32(reg_loss) + np.float32(GAMMA) * norm_loss,
        dtype=np.float32,
    )
